# revision 1
# baseline (speedup 1.0000x reference)
"""Bidirectional leaky-ESN (B=8,T=2048,D=64,H=1024,O=16) on 8 TRN2 NeuronCores.

Strategy
--------
The recurrence  h_t = 0.1 h_{t-1} + 0.9 tanh(u_proj_t + h_{t-1} W^T)  is a
contraction (leak 0.9, spectral radius 0.9; measured decay ~0.56/step), so
time can be chunked with a short washout: each of the 2 directions x 8
batches is split into C=64 chunks of L=32 steps; every chunk runs
independently from state 0 starting WASH=12 steps early.  Initial-condition
error decays below the bf16 compute floor (~2e-4 vs ~3.5e-3 measured in
simulation against an fp64 oracle).

This turns 2*2048 serial steps into L+WASH=44 steps over 1024 parallel
sequences.  Sharding: cores 0-3 forward direction (batches 2k,2k+1),
cores 4-7 backward - 128 sequences per core = full PE partition width,
single w_out section per core.

With s := h/0.9 the leak folds into W' = 0.9 W and w_out'' = 0.9 w_out:
    s_k = 0.1 s_{k-1} + tanh(u_proj_k + W' s_{k-1}),   h = 0.9 s.
State is kept transposed (H on partitions: 8 tiles [128,128] bf16,
sequences on the free dim).  Per step: 8 u-injection matmuls (K=65,
w_in|w_bias augmented, streamed input prearranged host-side) + 64
W'^T-stationary matmuls accumulate pre-activations into PSUM (8 banks,
one per H-tile); ScalarE tanh -> z (bf16); VectorE computes
s_new = 0.1*s + z (tensor_scalar + tensor_add).  The matmul stream runs
at the issue-rate floor (~56ns per LDWEIGHTS/MATMUL pair, N=128).

States for the L real steps land in a store; readout matmul groups
(q_m = w_out''^T s_m, [16 x 128] PSUM) are interleaved into the loop as
their states become ready, with PSUM->SBUF copies and per-group output
DMAs overlapped.  Host reassembles fwd+bwd+bias into [B,T,O].
"""

import numpy as np
import ml_dtypes

bf16 = ml_dtypes.bfloat16

B, T, D, H, O = 8, 2048, 64, 1024, 16
A = 0.9           # leaky rate
C = 64            # chunks per (batch, direction)
L = T // C        # 32 steps of real output per chunk
WASH = 8          # washout steps
STEPS = L + WASH
NCORES = 8
NI = H // 128     # 8 partition tiles of H
KAUG = D + 1      # 65: input dim + bias indicator row

_cached = {}


def _build_program():
    import concourse.bacc as bacc
    import concourse.mybir as mybir
    from concourse.tile import TileContext

    dt = mybir.dt
    nc = bacc.Bacc(trn_type="TRN2", target_bir_lowering=False, debug=False)

    # wTall[p, j*1024+i] = W'^T[j*128+p, i]: one DMA, 16KB contiguous/partition
    wT_d = nc.dram_tensor("wT", [128, NI * H], dt.bfloat16, kind="ExternalInput").ap()
    winT_d = nc.dram_tensor("winT", [KAUG, H], dt.bfloat16, kind="ExternalInput").ap()
    woutT_d = nc.dram_tensor("woutT", [128, NI * O], dt.bfloat16, kind="ExternalInput").ap()
    vbuf_d = nc.dram_tensor("vbuf", [KAUG, STEPS * 128], dt.bfloat16, kind="ExternalInput").ap()
    qout_d = nc.dram_tensor("qout", [O, L * 128], dt.float32, kind="ExternalOutput").ap()

    with TileContext(nc) as tc:
        _body(tc, mybir, wT_d, winT_d, woutT_d, vbuf_d, qout_d)
    nc.compile()
    return nc


def _body(tc, mybir, wT_d, winT_d, woutT_d, vbuf_d, qout_d):
    dt = mybir.dt
    nc = tc.nc
    Tanh = mybir.ActivationFunctionType.Tanh

    with (
        tc.tile_pool(name="const", bufs=1) as constp,
        tc.tile_pool(name="state", bufs=4) as statep,
        tc.tile_pool(name="zp", bufs=3) as zp,
        tc.tile_pool(name="tp", bufs=3) as tp,
        tc.tile_pool(name="store", bufs=1) as storep,
        tc.tile_pool(name="stage", bufs=1) as stagep,
        tc.tile_pool(name="pre", bufs=1, space="PSUM") as prep,
    ):
        # ---- prologue: load weights + all per-step inputs ----
        winT_sb = constp.tile([KAUG, H], dt.bfloat16, tag="winT", name="winT")
        nc.sync.dma_start(winT_sb[:], winT_d[:])
        vbuf_sb = constp.tile([KAUG, STEPS * 128], dt.bfloat16, tag="vbuf", name="vbuf")
        nc.sync.dma_start(vbuf_sb[:], vbuf_d[:])
        wT_sb = constp.tile([128, NI * H], dt.bfloat16, tag="wT", name="wT")
        nc.sync.dma_start(wT_sb[:], wT_d[:])
        woutT_sb = constp.tile([128, NI * O], dt.bfloat16, tag="woutT", name="woutT")
        nc.sync.dma_start(woutT_sb[:], woutT_d[:])

        store_sb = [storep.tile([128, L * 128], dt.bfloat16, tag=f"st{i}", name=f"st{i}")
                    for i in range(NI)]
        stage_sb = stagep.tile([O, L * 128], dt.float32, tag="stage", name="stage")

        def readout_group(g):
            """q_m = w_out''^T s_m for slots m in [4g, 4g+4): 32 MMs + copy + DMA."""
            pr = prep.tile([O, 512], dt.float32, tag=f"pre{g % NI}", name=f"pr_{g}")
            for mm in range(4):
                m = g * 4 + mm
                for i in range(NI):
                    nc.tensor.matmul(pr[:, mm * 128:(mm + 1) * 128],
                                     woutT_sb[:, i * O:(i + 1) * O],
                                     store_sb[i][:, m * 128:(m + 1) * 128],
                                     start=(i == 0), stop=(i == NI - 1))
            nc.scalar.copy(stage_sb[:, g * 512:(g + 1) * 512], pr)
            nc.sync.dma_start(qout_d[:, g * 512:(g + 1) * 512],
                              stage_sb[:, g * 512:(g + 1) * 512])

        # ---- serial recurrence, all 128 sequences in lockstep ----
        s_prev = None
        for k in range(STEPS):
            vk = vbuf_sb[:, k * 128:(k + 1) * 128]
            if k >= WASH:
                m = k - WASH
                s_cur = [store_sb[i][:, m * 128:(m + 1) * 128] for i in range(NI)]
            else:
                s_cur = [statep.tile([128, 128], dt.bfloat16, tag=f"s{i}", name=f"s{i}_{k}")
                         for i in range(NI)]
            # hoist u-injection for banks 0-3 only: their WAR (prev step's
            # tanh on that bank) cleared early, so these are safe boundary
            # filler that defers group 0's last state-dependent matmul past
            # the tanh->update chain latency
            pres = {}
            if k > 0:
                for i in range(4):
                    pres[i] = prep.tile([128, 128], dt.float32, tag=f"pre{i}",
                                        name=f"pre{i}_{k}")
                    nc.tensor.matmul(pres[i], winT_sb[:, i * 128:(i + 1) * 128], vk,
                                     start=True, stop=False)
            for i in range(NI):
                if i in pres:
                    pre = pres[i]
                else:
                    pre = prep.tile([128, 128], dt.float32, tag=f"pre{i}", name=f"pre{i}_{k}")
                    nc.tensor.matmul(pre, winT_sb[:, i * 128:(i + 1) * 128], vk,
                                     start=True, stop=(k == 0))
                if k > 0:
                    for j in range(NI):
                        nc.tensor.matmul(pre, wT_sb[:, j * H + i * 128:j * H + (i + 1) * 128],
                                         s_prev[j], start=False, stop=(j == NI - 1))
                if k == 0:
                    nc.scalar.activation(s_cur[i], pre, Tanh)
                else:
                    z = zp.tile([128, 128], dt.bfloat16, tag=f"z{i}", name=f"z{i}_{k}")
                    nc.scalar.activation(z, pre, Tanh)
                    # s_new = (s_prev * 0.1) + z
                    t01 = tp.tile([128, 128], dt.bfloat16, tag=f"t{i}", name=f"t{i}_{k}")
                    nc.vector.tensor_scalar_mul(t01, s_prev[i], 0.1)
                    nc.vector.tensor_add(s_cur[i], t01, z)
            s_prev = s_cur
            # interleave readout as soon as a 4-slot group of states is complete
            mdone = k - WASH + 1
            if mdone >= 4 and mdone % 4 == 0:
                readout_group(mdone // 4 - 1)


def _prep_inputs(u, w, w_in, w_bias, w_out):
    """Host-side prep: per-core input maps (bf16 except the f32 output)."""
    WT = np.ascontiguousarray((A * w).T).astype(np.float32)               # [j, i]
    wTall = np.ascontiguousarray(
        WT.reshape(NI, 128, H).transpose(1, 0, 2).reshape(128, NI * H)).astype(bf16)
    winT = np.ascontiguousarray(
        np.concatenate([w_in, w_bias[:, None]], axis=1).T).astype(bf16)   # [65, H]
    in_maps = []
    for core in range(NCORES):
        d = core // 4                       # 0 fwd, 1 bwd
        w2 = (A * w_out[1 + d * H:1 + (d + 1) * H, :]).astype(np.float32)  # [H, O]
        woutT = np.ascontiguousarray(
            w2.reshape(NI, 128, O).transpose(1, 0, 2).reshape(128, NI * O)).astype(bf16)
        v = np.zeros((STEPS, KAUG, 128), np.float32)
        ks = np.arange(STEPS)
        for b_loc in range(2):
            b = 2 * (core % 4) + b_loc
            ud = u[b] if d == 0 else u[b, ::-1]
            for c in range(C):
                ts = c * L - WASH + ks
                valid = ts >= 0
                s_idx = b_loc * C + c
                v[valid, :D, s_idx] = ud[ts[valid]]
                v[valid, D, s_idx] = 1.0
        vbuf = np.ascontiguousarray(
            v.transpose(1, 0, 2).reshape(KAUG, STEPS * 128)).astype(bf16)
        in_maps.append({"wT": wTall, "winT": winT, "woutT": woutT, "vbuf": vbuf})
    return in_maps


def _assemble(results, w_out):
    y = np.zeros((B, T, O), np.float32)
    for core in range(NCORES):
        q = np.asarray(results[core]["qout"], np.float32).reshape(O, L, 128)
        d = core // 4
        for b_loc in range(2):
            b = 2 * (core % 4) + b_loc
            qq = q[:, :, b_loc * C:(b_loc + 1) * C]       # [O, L(m), C(c)]
            tmp = qq.transpose(2, 1, 0).reshape(T, O)     # t = c*L + m
            if d == 0:
                y[b] += tmp
            else:
                y[b, ::-1] += tmp
    y += w_out[0][None, None, :].astype(np.float32)
    return y


def kernel(u, w, w_in, w_bias, w_out):
    from concourse.bass_utils import run_bass_kernel_spmd

    u = np.asarray(u, np.float32)
    w = np.asarray(w, np.float32)
    w_in = np.asarray(w_in, np.float32)
    w_bias = np.asarray(w_bias, np.float32)
    w_out = np.asarray(w_out, np.float32)

    if "nc" not in _cached:
        _cached["nc"] = _build_program()
    nc = _cached["nc"]
    in_maps = _prep_inputs(u, w, w_in, w_bias, w_out)
    res = run_bass_kernel_spmd(nc, in_maps, list(range(NCORES)))
    return _assemble(res.results, w_out)



# revision 4
# speedup vs baseline: 1.0111x; 1.0111x over previous
"""Bidirectional leaky-ESN (B=8,T=2048,D=64,H=1024,O=16) on 8 TRN2 NeuronCores.

Strategy (v2)
-------------
Chunked-washout time parallelism as v1: recurrence is a contraction
(~0.56/step), so each of 16 (batch x direction) chains is split into C=64
chunks of L=32 steps run independently with a WASH=6 washout.  128
sequences per core; state transposed (H on partitions, 8 bf16 tiles
[128,128]); per step 8 u-injection matmuls (K=65) + 64 W'-stationary
matmuls accumulate into PSUM; ScalarE tanh; DVE leaky update.

v2 changes vs the 204us baseline:
- step 0 (tanh of the input projection only) is computed on the host and
  shipped as an s0 input: removes a serial ScalarE chain + 8 pairs.
- DMA split and ordered (winT, vbuf head, wT, s0, vbuf tail, woutT) so
  the u-inject stream starts ~7us earlier; ~NWARM dummy matmul pairs run
  during the DMA wait so the PE is at full p-state (2.4GHz, issue floor
  ~56ns/pair) when real work arrives instead of ramping at 1.2GHz.
- pre-activation PSUM tiles double-buffered; each step's u-injects are
  emitted after the previous step's groups, so no W matmul ever waits on
  the tanh/DVE chain (kills the ~168ns/step boundary stall).
- WASH 8 -> 6 (measured IC error well under the 2e-2 gate).
- readout runs once at the end, col-tiled: w_out^T tiles (M=16) go to
  4 concurrent 32-column PE groups (tile_position), 2 accumulated MMs per
  group -> ~4x fewer PE-serialized columns than the v1 readout; 4 partial
  strips land at PSUM partitions {0,32,64,96}+0..15 and are summed on the
  host; staged to bf16 for a 1MB output DMA.
"""

import numpy as np
import ml_dtypes

bf16 = ml_dtypes.bfloat16

B, T, D, H, O = 8, 2048, 64, 1024, 16
A = 0.9           # leaky rate
C = 64            # chunks per (batch, direction)
L = T // C        # 32 steps of real output per chunk
WASH = 6          # washout steps (step 0 runs on host)
STEPS = L + WASH
NCORES = 8
NI = H // 128     # 8 partition tiles of H
KAUG = D + 1      # 65: input dim + bias indicator row
NWARM = 64        # dummy PE warmup pairs during the DMA wait
VHEAD = 8         # steps of vbuf in the head DMA

_cached = {}


def _build_program():
    import concourse.bacc as bacc
    import concourse.mybir as mybir
    from concourse.tile import TileContext

    dt = mybir.dt
    nc = bacc.Bacc(trn_type="TRN2", target_bir_lowering=False, debug=False)

    # wTall[p, j*1024+i] = W'^T[j*128+p, i]: one DMA, contiguous/partition
    wT_d = nc.dram_tensor("wT", [128, NI * H], dt.bfloat16, kind="ExternalInput").ap()
    winT_d = nc.dram_tensor("winT", [KAUG, H], dt.bfloat16, kind="ExternalInput").ap()
    woutT_d = nc.dram_tensor("woutT", [128, NI * O], dt.bfloat16, kind="ExternalInput").ap()
    s0_d = nc.dram_tensor("s0", [128, NI * 128], dt.bfloat16, kind="ExternalInput").ap()
    vA_d = nc.dram_tensor("vA", [KAUG, VHEAD * 128], dt.bfloat16, kind="ExternalInput").ap()
    vB_d = nc.dram_tensor("vB", [KAUG, (STEPS - 1 - VHEAD) * 128], dt.bfloat16,
                          kind="ExternalInput").ap()
    qout_d = nc.dram_tensor("qout", [128, L * 128], dt.bfloat16, kind="ExternalOutput").ap()

    with TileContext(nc) as tc:
        _body(tc, mybir, wT_d, winT_d, woutT_d, s0_d, vA_d, vB_d, qout_d)
    nc.compile()
    return nc


def _body(tc, mybir, wT_d, winT_d, woutT_d, s0_d, vA_d, vB_d, qout_d):
    dt = mybir.dt
    nc = tc.nc
    Tanh = mybir.ActivationFunctionType.Tanh

    with (
        tc.tile_pool(name="const", bufs=1) as constp,
        tc.tile_pool(name="state", bufs=3) as statep,
        tc.tile_pool(name="zp", bufs=3) as zp,
        tc.tile_pool(name="tp", bufs=3) as tp,
        tc.tile_pool(name="store", bufs=1) as storep,
        tc.tile_pool(name="stage", bufs=1) as stagep,
        tc.tile_pool(name="pre", bufs=2, space="PSUM") as prep,
        tc.tile_pool(name="ro", bufs=2, space="PSUM") as rop,
        tc.tile_pool(name="dm", bufs=1, space="PSUM") as dmp,
    ):
        # ---- dummy warmup tile (zeroed by DVE; garbage-safe either way) ----
        dummy = constp.tile([128, 128], dt.bfloat16, tag="dummy", name="dummy")
        nc.vector.memset(dummy[:], 0.0)

        # ---- input DMAs, ordered for earliest possible compute start ----
        winT_sb = constp.tile([KAUG, H], dt.bfloat16, tag="winT", name="winT")
        nc.sync.dma_start(winT_sb[:], winT_d[:])
        vA_sb = constp.tile([KAUG, VHEAD * 128], dt.bfloat16, tag="vA", name="vA")
        nc.sync.dma_start(vA_sb[:], vA_d[:])
        wT_sb = constp.tile([128, NI * H], dt.bfloat16, tag="wT", name="wT")
        nc.sync.dma_start(wT_sb[:], wT_d[:])
        s0_sb = constp.tile([128, NI * 128], dt.bfloat16, tag="s0", name="s0")
        nc.sync.dma_start(s0_sb[:], s0_d[:])
        vB_sb = constp.tile([KAUG, (STEPS - 1 - VHEAD) * 128], dt.bfloat16,
                            tag="vB", name="vB")
        nc.sync.dma_start(vB_sb[:], vB_d[:])
        woutT_sb = constp.tile([128, NI * O], dt.bfloat16, tag="woutT", name="woutT")
        nc.sync.dma_start(woutT_sb[:], woutT_d[:])

        store_sb = [storep.tile([128, L * 128], dt.bfloat16, tag=f"st{i}", name=f"st{i}")
                    for i in range(NI)]
        stage_sb = stagep.tile([128, L * 128], dt.bfloat16, tag="stage", name="stage")

        def vk(k):
            """input column block for kernel step k (k>=1)."""
            if k <= VHEAD:
                return vA_sb[:, (k - 1) * 128:k * 128]
            return vB_sb[:, (k - 1 - VHEAD) * 128:(k - VHEAD) * 128]

        # ---- PE warmup: independent dummy pairs run during the DMA wait ----
        dmps = dmp.tile([128, 128], dt.float32, tag="dm", name="dm")
        for w in range(NWARM):
            nc.tensor.matmul(dmps, dummy[:], dummy[:], start=True, stop=True)

        def u_inject(k):
            """returns the 8 pre-activation slices for step k (2 banks,
            4 column-packed [128,128] regions each), u-injection issued."""
            pA = prep.tile([128, 512], dt.float32, tag="preA", name=f"preA_{k}")
            pB = prep.tile([128, 512], dt.float32, tag="preB", name=f"preB_{k}")
            pres = []
            for i in range(NI):
                bank = pA if i < 4 else pB
                pre = bank[:, (i % 4) * 128:(i % 4 + 1) * 128]
                nc.tensor.matmul(pre, winT_sb[:, i * 128:(i + 1) * 128], vk(k),
                                 start=True, stop=False)
                pres.append(pre)
            return pres

        # pre-inject steps 1 and 2 (fills PSUM double buffer)
        preQ = [u_inject(1), u_inject(2)]

        s_prev = [s0_sb[:, i * 128:(i + 1) * 128] for i in range(NI)]
        for k in range(1, STEPS):
            pres = preQ.pop(0)
            s_cur = []
            for i in range(NI):
                pre = pres[i]
                for j in range(NI):
                    nc.tensor.matmul(pre, wT_sb[:, j * H + i * 128:j * H + (i + 1) * 128],
                                     s_prev[j], start=False, stop=(j == NI - 1))
                if k >= WASH:
                    m = k - WASH
                    sc = store_sb[i][:, m * 128:(m + 1) * 128]
                else:
                    sc = statep.tile([128, 128], dt.bfloat16, tag=f"s{i}", name=f"s{i}_{k}")
                z = zp.tile([128, 128], dt.bfloat16, tag=f"z{i}", name=f"z{i}_{k}")
                nc.scalar.activation(z, pre, Tanh)
                t01 = tp.tile([128, 128], dt.bfloat16, tag=f"t{i}", name=f"t{i}_{k}")
                nc.vector.tensor_scalar_mul(t01, s_prev[i], 0.1)
                nc.vector.tensor_add(sc, t01, z)
                s_cur.append(sc)
            # u-injects for step k+2 land between step k and k+1 on the PE
            # queue: boundary filler + no W matmul ever waits on tanh/DVE
            if k + 2 < STEPS:
                preQ.append(u_inject(k + 2))
            s_prev = s_cur

        # ---- readout, col-tiled: 4 concurrent 32-col groups, M=16 each ----
        for g in range(8):  # 4-slot groups of 512 columns
            ro = rop.tile([128, 512], dt.float32, tag="ro", name=f"ro{g}")
            for i in range(NI):
                cg = 32 * (i % 4)
                nc.tensor.matmul(ro[cg:cg + O, :],
                                 woutT_sb[:, i * O:(i + 1) * O],
                                 store_sb[i][:, g * 512:(g + 1) * 512],
                                 start=(i < 4), stop=(i >= 4),
                                 tile_position=(0, cg))
            nc.scalar.copy(stage_sb[:, g * 512:(g + 1) * 512], ro)
            if g in (3, 7):
                lo = 0 if g == 3 else 2048
                nc.sync.dma_start(qout_d[:, lo:lo + 2048],
                                  stage_sb[:, lo:lo + 2048])


def _prep_inputs(u, w, w_in, w_bias, w_out):
    """Host-side prep: per-core input maps (bf16 except host-summed output)."""
    WT = np.ascontiguousarray((A * w).T).astype(np.float32)               # [j, i]
    wTall = np.ascontiguousarray(
        WT.reshape(NI, 128, H).transpose(1, 0, 2).reshape(128, NI * H)).astype(bf16)
    win_full = np.concatenate([w_in, w_bias[:, None]], axis=1)            # [H, 65]
    winT = np.ascontiguousarray(win_full.T).astype(bf16)                  # [65, H]
    in_maps = []
    for core in range(NCORES):
        d = core // 4                       # 0 fwd, 1 bwd
        w2 = (A * w_out[1 + d * H:1 + (d + 1) * H, :]).astype(np.float32)  # [H, O]
        woutT = np.ascontiguousarray(
            w2.reshape(NI, 128, O).transpose(1, 0, 2).reshape(128, NI * O)).astype(bf16)
        v = np.zeros((STEPS, KAUG, 128), np.float32)
        ks = np.arange(STEPS)
        for b_loc in range(2):
            b = 2 * (core % 4) + b_loc
            ud = u[b] if d == 0 else u[b, ::-1]
            for c in range(C):
                ts = c * L - WASH + ks
                valid = ts >= 0
                s_idx = b_loc * C + c
                v[valid, :D, s_idx] = ud[ts[valid]]
                v[valid, D, s_idx] = 1.0
        # step 0 on host: s0 = tanh(u_proj(t0)) (zero where t0 < 0)
        up0 = np.tanh(v[0].T.astype(np.float32) @ win_full.T.astype(np.float32))  # [128, H]
        s0 = np.ascontiguousarray(
            up0.T.reshape(NI, 128, 128).transpose(1, 0, 2).reshape(128, NI * 128)
        ).astype(bf16)
        vsteps = v[1:].transpose(1, 0, 2)     # [KAUG, STEPS-1, 128]
        vA = np.ascontiguousarray(
            vsteps[:, :VHEAD].reshape(KAUG, VHEAD * 128)).astype(bf16)
        vB = np.ascontiguousarray(
            vsteps[:, VHEAD:].reshape(KAUG, (STEPS - 1 - VHEAD) * 128)).astype(bf16)
        in_maps.append({"wT": wTall, "winT": winT, "woutT": woutT,
                        "s0": s0, "vA": vA, "vB": vB})
    return in_maps


def _assemble(results, w_out):
    y = np.zeros((B, T, O), np.float32)
    for core in range(NCORES):
        qs = np.asarray(results[core]["qout"])                 # [128, L*128] bf16
        q = np.zeros((O, L * 128), np.float32)
        for gp in range(4):
            q += qs[32 * gp:32 * gp + O].astype(np.float32)
        q = q.reshape(O, L, 128)
        d = core // 4
        for b_loc in range(2):
            b = 2 * (core % 4) + b_loc
            qq = q[:, :, b_loc * C:(b_loc + 1) * C]       # [O, L(m), C(c)]
            tmp = qq.transpose(2, 1, 0).reshape(T, O)     # t = c*L + m
            if d == 0:
                y[b] += tmp
            else:
                y[b, ::-1] += tmp
    y += w_out[0][None, None, :].astype(np.float32)
    return y


def kernel(u, w, w_in, w_bias, w_out):
    from concourse.bass_utils import run_bass_kernel_spmd

    u = np.asarray(u, np.float32)
    w = np.asarray(w, np.float32)
    w_in = np.asarray(w_in, np.float32)
    w_bias = np.asarray(w_bias, np.float32)
    w_out = np.asarray(w_out, np.float32)

    if "nc" not in _cached:
        _cached["nc"] = _build_program()
    nc = _cached["nc"]
    in_maps = _prep_inputs(u, w, w_in, w_bias, w_out)
    res = run_bass_kernel_spmd(nc, in_maps, list(range(NCORES)))
    return _assemble(res.results, w_out)


# revision 15
# speedup vs baseline: 1.0624x; 1.0508x over previous
"""Bidirectional leaky-ESN (B=8,T=2048,D=64,H=1024,O=16) on 8 TRN2 NeuronCores.

Strategy (v2)
-------------
Chunked-washout time parallelism as v1: recurrence is a contraction
(~0.56/step), so each of 16 (batch x direction) chains is split into C=64
chunks of L=32 steps run independently with a WASH=6 washout.  128
sequences per core; state transposed (H on partitions, 8 bf16 tiles
[128,128]); per step 8 u-injection matmuls (K=65) + 64 W'-stationary
matmuls accumulate into PSUM; ScalarE tanh; DVE leaky update.

v2 changes vs the 204us baseline:
- step 0 (tanh of the input projection only) is computed on the host and
  shipped as an s0 input: removes a serial ScalarE chain + 8 pairs.
- DMA split and ordered (winT, vbuf head, wT, s0, vbuf tail, woutT) so
  the u-inject stream starts ~7us earlier; ~NWARM dummy matmul pairs run
  during the DMA wait so the PE is at full p-state (2.4GHz, issue floor
  ~56ns/pair) when real work arrives instead of ramping at 1.2GHz.
- pre-activation PSUM tiles double-buffered; each step's u-injects are
  emitted after the previous step's groups, so no W matmul ever waits on
  the tanh/DVE chain (kills the ~168ns/step boundary stall).
- WASH 8 -> 6 (measured IC error well under the 2e-2 gate).
- readout runs once at the end, col-tiled: w_out^T tiles (M=16) go to
  4 concurrent 32-column PE groups (tile_position), 2 accumulated MMs per
  group -> ~4x fewer PE-serialized columns than the v1 readout; 4 partial
  strips land at PSUM partitions {0,32,64,96}+0..15 and are summed on the
  host; staged to bf16 for a 1MB output DMA.
"""

import numpy as np
import ml_dtypes

bf16 = ml_dtypes.bfloat16

B, T, D, H, O = 8, 2048, 64, 1024, 16
A = 0.9           # leaky rate
C = 64            # chunks per (batch, direction)
L = T // C        # 32 steps of real output per chunk
WASH = 6          # washout steps (step 0 runs on host)
STEPS = L + WASH
NCORES = 8
NI = H // 128     # 8 partition tiles of H
KAUG = D + 1      # 65: input dim + bias indicator row
NWARM = 30        # dummy PE warmup matmuls (N=512) during the DMA wait
VHEAD = 8         # steps of vbuf in the head DMA

_cached = {}


def _build_program():
    import concourse.bacc as bacc
    import concourse.mybir as mybir
    from concourse.tile import TileContext

    dt = mybir.dt
    nc = bacc.Bacc(trn_type="TRN2", target_bir_lowering=False, debug=False)

    # wTall[p, j*1024+i] = W'^T[j*128+p, i]: one DMA, contiguous/partition
    wT_d = nc.dram_tensor("wT", [128, NI * H], dt.bfloat16, kind="ExternalInput").ap()
    winT_d = nc.dram_tensor("winT", [KAUG, H], dt.bfloat16, kind="ExternalInput").ap()
    woutT_d = nc.dram_tensor("woutT", [128, NI * O], dt.bfloat16, kind="ExternalInput").ap()
    s0_d = nc.dram_tensor("s0", [128, NI * 128], dt.bfloat16, kind="ExternalInput").ap()
    vA_d = nc.dram_tensor("vA", [KAUG, VHEAD * 128], dt.bfloat16, kind="ExternalInput").ap()
    vB_d = nc.dram_tensor("vB", [KAUG, (STEPS - 1 - VHEAD) * 128], dt.bfloat16,
                          kind="ExternalInput").ap()
    qout_d = nc.dram_tensor("qout", [O, L * 128], dt.bfloat16, kind="ExternalOutput").ap()

    with TileContext(nc) as tc:
        _body(tc, mybir, wT_d, winT_d, woutT_d, s0_d, vA_d, vB_d, qout_d)
    nc.compile()
    return nc


def _body(tc, mybir, wT_d, winT_d, woutT_d, s0_d, vA_d, vB_d, qout_d):
    dt = mybir.dt
    nc = tc.nc
    Tanh = mybir.ActivationFunctionType.Tanh

    with (
        tc.tile_pool(name="const", bufs=1) as constp,
        tc.tile_pool(name="state", bufs=3) as statep,
        tc.tile_pool(name="zp", bufs=3) as zp,
        tc.tile_pool(name="tp", bufs=3) as tp,
        tc.tile_pool(name="store", bufs=1) as storep,
        tc.tile_pool(name="stage", bufs=1) as stagep,
        tc.tile_pool(name="pre", bufs=1, space="PSUM") as prep,
    ):
        # ---- dummy warmup tile (zeroed by DVE; garbage-safe either way).
        # K=1/M=1 stationary + N=512 moving: keeps the PE p-state ramping
        # during the DMA wait at ~5GB/s of SBUF traffic so the input DMAs
        # keep full bandwidth.
        dummy = constp.tile([1, 512], dt.bfloat16, tag="dummy", name="dummy")
        nc.vector.memset(dummy[:], 0.0)

        # ---- input DMAs, ordered for earliest possible compute start ----
        winT_sb = constp.tile([KAUG, H], dt.bfloat16, tag="winT", name="winT")
        nc.sync.dma_start(winT_sb[:], winT_d[:])
        vA_sb = constp.tile([KAUG, VHEAD * 128], dt.bfloat16, tag="vA", name="vA")
        nc.sync.dma_start(vA_sb[:], vA_d[:])
        wT_sb = constp.tile([128, NI * H], dt.bfloat16, tag="wT", name="wT")
        nc.sync.dma_start(wT_sb[:], wT_d[:])
        s0_sb = constp.tile([128, NI * 128], dt.bfloat16, tag="s0", name="s0")
        nc.sync.dma_start(s0_sb[:], s0_d[:])
        vB_sb = constp.tile([KAUG, (STEPS - 1 - VHEAD) * 128], dt.bfloat16,
                            tag="vB", name="vB")
        nc.sync.dma_start(vB_sb[:], vB_d[:])
        woutT_sb = constp.tile([128, NI * O], dt.bfloat16, tag="woutT", name="woutT")
        nc.sync.dma_start(woutT_sb[:], woutT_d[:])

        store_sb = [storep.tile([128, L * 128], dt.bfloat16, tag=f"st{i}", name=f"st{i}")
                    for i in range(NI)]
        stage_sb = stagep.tile([O, L * 128], dt.bfloat16, tag="stage", name="stage")

        def vk(k):
            """input column block for kernel step k (k>=1)."""
            if k <= VHEAD:
                return vA_sb[:, (k - 1) * 128:k * 128]
            return vB_sb[:, (k - 1 - VHEAD) * 128:(k - VHEAD) * 128]

        # ---- PE warmup: independent dummy pairs run during the DMA wait ----
        dmps = prep.tile([1, 512], dt.float32, tag="pre0", name="dm")
        for w in range(NWARM):
            nc.tensor.matmul(dmps, dummy[0:1, 0:1], dummy[0:1, :],
                             start=True, stop=True)

        def u_inject(k):
            """returns the 8 pre-activation tiles for step k (one PSUM bank
            each; a bank is only ever read after its accumulation closes)."""
            pres = []
            for i in range(NI):
                pre = prep.tile([128, 128], dt.float32, tag=f"pre{i}", name=f"pre{i}_{k}")
                nc.tensor.matmul(pre, winT_sb[:, i * 128:(i + 1) * 128], vk(k),
                                 start=True, stop=False)
                pres.append(pre)
            return pres

        preQ = [u_inject(1)]

        s_prev = [s0_sb[:, i * 128:(i + 1) * 128] for i in range(NI)]
        for k in range(1, STEPS):
            pres = preQ.pop(0)
            s_cur = []
            for i in range(NI):
                pre = pres[i]
                for j in range(NI):
                    nc.tensor.matmul(pre, wT_sb[:, j * H + i * 128:j * H + (i + 1) * 128],
                                     s_prev[j], start=False, stop=(j == NI - 1))
                if k >= WASH:
                    m = k - WASH
                    sc = store_sb[i][:, m * 128:(m + 1) * 128]
                else:
                    sc = statep.tile([128, 128], dt.bfloat16, tag=f"s{i}", name=f"s{i}_{k}")
                z = zp.tile([128, 128], dt.bfloat16, tag=f"z{i}", name=f"z{i}_{k}")
                nc.scalar.activation(z, pre, Tanh)
                t01 = tp.tile([128, 128], dt.bfloat16, tag=f"t{i}", name=f"t{i}_{k}")
                nc.vector.tensor_scalar_mul(t01, s_prev[i], 0.1)
                nc.vector.tensor_add(sc, t01, z)
                s_cur.append(sc)
            # u-injects for step k+1 land at the step boundary on the PE
            # queue: boundary filler; bank i's WAR (tanh_i of step k) has
            # cleared by the time inject i issues
            if k + 1 < STEPS:
                preQ.append(u_inject(k + 1))
            s_prev = s_cur

        # ---- readout: accumulate all 8 H-tiles into [16,512] per group ----
        for g in range(8):  # 4-slot groups of 512 columns
            ro = prep.tile([O, 512], dt.float32, tag=f"pre{g}", name=f"ro{g}")
            for i in range(NI):
                nc.tensor.matmul(ro[:, :],
                                 woutT_sb[:, i * O:(i + 1) * O],
                                 store_sb[i][:, g * 512:(g + 1) * 512],
                                 start=(i == 0), stop=(i == NI - 1))
            nc.scalar.copy(stage_sb[:, g * 512:(g + 1) * 512], ro)
            if g in (3, 7):
                lo = 0 if g == 3 else 2048
                nc.sync.dma_start(qout_d[:, lo:lo + 2048],
                                  stage_sb[:, lo:lo + 2048])


def _prep_inputs(u, w, w_in, w_bias, w_out):
    """Host-side prep: per-core input maps (bf16 except host-summed output)."""
    WT = np.ascontiguousarray((A * w).T).astype(np.float32)               # [j, i]
    wTall = np.ascontiguousarray(
        WT.reshape(NI, 128, H).transpose(1, 0, 2).reshape(128, NI * H)).astype(bf16)
    win_full = np.concatenate([w_in, w_bias[:, None]], axis=1)            # [H, 65]
    winT = np.ascontiguousarray(win_full.T).astype(bf16)                  # [65, H]
    in_maps = []
    for core in range(NCORES):
        d = core // 4                       # 0 fwd, 1 bwd
        w2 = (A * w_out[1 + d * H:1 + (d + 1) * H, :]).astype(np.float32)  # [H, O]
        woutT = np.ascontiguousarray(
            w2.reshape(NI, 128, O).transpose(1, 0, 2).reshape(128, NI * O)).astype(bf16)
        v = np.zeros((STEPS, KAUG, 128), np.float32)
        ks = np.arange(STEPS)
        for b_loc in range(2):
            b = 2 * (core % 4) + b_loc
            ud = u[b] if d == 0 else u[b, ::-1]
            for c in range(C):
                ts = c * L - WASH + ks
                valid = ts >= 0
                s_idx = b_loc * C + c
                v[valid, :D, s_idx] = ud[ts[valid]]
                v[valid, D, s_idx] = 1.0
        # step 0 on host: s0 = tanh(u_proj(t0)) (zero where t0 < 0)
        up0 = np.tanh(v[0].T.astype(np.float32) @ win_full.T.astype(np.float32))  # [128, H]
        s0 = np.ascontiguousarray(
            up0.T.reshape(NI, 128, 128).transpose(1, 0, 2).reshape(128, NI * 128)
        ).astype(bf16)
        vsteps = v[1:].transpose(1, 0, 2)     # [KAUG, STEPS-1, 128]
        vA = np.ascontiguousarray(
            vsteps[:, :VHEAD].reshape(KAUG, VHEAD * 128)).astype(bf16)
        vB = np.ascontiguousarray(
            vsteps[:, VHEAD:].reshape(KAUG, (STEPS - 1 - VHEAD) * 128)).astype(bf16)
        in_maps.append({"wT": wTall, "winT": winT, "woutT": woutT,
                        "s0": s0, "vA": vA, "vB": vB})
    return in_maps


def _assemble(results, w_out):
    y = np.zeros((B, T, O), np.float32)
    for core in range(NCORES):
        q = np.asarray(results[core]["qout"]).astype(np.float32).reshape(O, L, 128)
        d = core // 4
        for b_loc in range(2):
            b = 2 * (core % 4) + b_loc
            qq = q[:, :, b_loc * C:(b_loc + 1) * C]       # [O, L(m), C(c)]
            tmp = qq.transpose(2, 1, 0).reshape(T, O)     # t = c*L + m
            if d == 0:
                y[b] += tmp
            else:
                y[b, ::-1] += tmp
    y += w_out[0][None, None, :].astype(np.float32)
    return y


def kernel(u, w, w_in, w_bias, w_out):
    from concourse.bass_utils import run_bass_kernel_spmd

    u = np.asarray(u, np.float32)
    w = np.asarray(w, np.float32)
    w_in = np.asarray(w_in, np.float32)
    w_bias = np.asarray(w_bias, np.float32)
    w_out = np.asarray(w_out, np.float32)

    if "nc" not in _cached:
        _cached["nc"] = _build_program()
    nc = _cached["nc"]
    in_maps = _prep_inputs(u, w, w_in, w_bias, w_out)
    res = run_bass_kernel_spmd(nc, in_maps, list(range(NCORES)))
    return _assemble(res.results, w_out)


# revision 26
# speedup vs baseline: 1.0902x; 1.0262x over previous
"""Bidirectional leaky-ESN (B=8,T=2048,D=64,H=1024,O=16) on 8 TRN2 NeuronCores.

Strategy (v2)
-------------
Chunked-washout time parallelism as v1: recurrence is a contraction
(~0.56/step), so each of 16 (batch x direction) chains is split into C=64
chunks of L=32 steps run independently with a WASH=6 washout.  128
sequences per core; state transposed (H on partitions, 8 bf16 tiles
[128,128]); per step 8 u-injection matmuls (K=65) + 64 W'-stationary
matmuls accumulate into PSUM; ScalarE tanh; DVE leaky update.

v2 changes vs the 204us baseline:
- step 0 (tanh of the input projection only) is computed on the host and
  shipped as an s0 input: removes a serial ScalarE chain + 8 pairs.
- DMA split and ordered (winT, vbuf head, wT, s0, vbuf tail, woutT) so
  the u-inject stream starts ~7us earlier; ~NWARM dummy matmul pairs run
  during the DMA wait so the PE is at full p-state (2.4GHz, issue floor
  ~56ns/pair) when real work arrives instead of ramping at 1.2GHz.
- pre-activation PSUM tiles double-buffered; each step's u-injects are
  emitted after the previous step's groups, so no W matmul ever waits on
  the tanh/DVE chain (kills the ~168ns/step boundary stall).
- WASH 8 -> 6 (measured IC error well under the 2e-2 gate).
- readout runs once at the end, col-tiled: w_out^T tiles (M=16) go to
  4 concurrent 32-column PE groups (tile_position), 2 accumulated MMs per
  group -> ~4x fewer PE-serialized columns than the v1 readout; 4 partial
  strips land at PSUM partitions {0,32,64,96}+0..15 and are summed on the
  host; staged to bf16 for a 1MB output DMA.
"""

import numpy as np
import ml_dtypes

bf16 = ml_dtypes.bfloat16

B, T, D, H, O = 8, 2048, 64, 1024, 16
A = 0.9           # leaky rate
C = 64            # chunks per (batch, direction)
L = T // C        # 32 steps of real output per chunk
WASH = 6          # washout steps (step 0 runs on host)
STEPS = L + WASH
NCORES = 8
NI = H // 128     # 8 partition tiles of H
KAUG = D + 1      # 65: input dim + bias indicator row
VHEAD = 8         # steps of vbuf in the head DMA

_cached = {}


def _build_program():
    import concourse.bacc as bacc
    import concourse.mybir as mybir
    from concourse.tile import TileContext

    dt = mybir.dt
    nc = bacc.Bacc(trn_type="TRN2", target_bir_lowering=False, debug=False)

    # wTall[p, j*1024+i] = W'^T[j*128+p, i]: split j 0-3 / 4-7 so step 1
    # can start on the first half while the second streams
    wT0_d = nc.dram_tensor("wT0", [128, 4 * H], dt.bfloat16, kind="ExternalInput").ap()
    wT1_d = nc.dram_tensor("wT1", [128, 4 * H], dt.bfloat16, kind="ExternalInput").ap()
    winT_d = nc.dram_tensor("winT", [KAUG, H], dt.bfloat16, kind="ExternalInput").ap()
    woutT_d = nc.dram_tensor("woutT", [128, NI * O], dt.bfloat16, kind="ExternalInput").ap()
    s0_d = nc.dram_tensor("s0", [128, NI * 128], dt.bfloat16, kind="ExternalInput").ap()
    vA_d = nc.dram_tensor("vA", [KAUG, VHEAD * 128], dt.bfloat16, kind="ExternalInput").ap()
    vB_d = nc.dram_tensor("vB", [KAUG, (STEPS - 1 - VHEAD) * 128], dt.bfloat16,
                          kind="ExternalInput").ap()
    qout_d = nc.dram_tensor("qout", [O, L * 128], dt.bfloat16, kind="ExternalOutput").ap()

    with TileContext(nc) as tc:
        _body(tc, mybir, wT0_d, wT1_d, winT_d, woutT_d, s0_d, vA_d, vB_d, qout_d)
    nc.compile()
    return nc


def _body(tc, mybir, wT0_d, wT1_d, winT_d, woutT_d, s0_d, vA_d, vB_d, qout_d):
    dt = mybir.dt
    nc = tc.nc
    Tanh = mybir.ActivationFunctionType.Tanh

    with (
        tc.tile_pool(name="const", bufs=1) as constp,
        tc.tile_pool(name="state", bufs=3) as statep,
        tc.tile_pool(name="zp", bufs=3) as zp,
        tc.tile_pool(name="tp", bufs=3) as tp,
        tc.tile_pool(name="store", bufs=1) as storep,
        tc.tile_pool(name="stage", bufs=1) as stagep,
        tc.tile_pool(name="pre", bufs=1, space="PSUM") as prep,
    ):
        # ---- input DMAs, ordered for earliest possible compute start ----
        winT_sb = constp.tile([KAUG, H], dt.bfloat16, tag="winT", name="winT")
        nc.sync.dma_start(winT_sb[:], winT_d[:])
        vA_sb = constp.tile([KAUG, VHEAD * 128], dt.bfloat16, tag="vA", name="vA")
        nc.sync.dma_start(vA_sb[:], vA_d[:])
        s0_sb = constp.tile([128, NI * 128], dt.bfloat16, tag="s0", name="s0")
        nc.sync.dma_start(s0_sb[:], s0_d[:])
        wT0_sb = constp.tile([128, 4 * H], dt.bfloat16, tag="wT0", name="wT0")
        nc.sync.dma_start(wT0_sb[:], wT0_d[:])
        wT1_sb = constp.tile([128, 4 * H], dt.bfloat16, tag="wT1", name="wT1")
        nc.sync.dma_start(wT1_sb[:], wT1_d[:])
        vB_sb = constp.tile([KAUG, (STEPS - 1 - VHEAD) * 128], dt.bfloat16,
                            tag="vB", name="vB")
        nc.sync.dma_start(vB_sb[:], vB_d[:])
        woutT_sb = constp.tile([128, NI * O], dt.bfloat16, tag="woutT", name="woutT")
        nc.sync.dma_start(woutT_sb[:], woutT_d[:])

        def wslice(j, i):
            if j < 4:
                return wT0_sb[:, j * H + i * 128:j * H + (i + 1) * 128]
            return wT1_sb[:, (j - 4) * H + i * 128:(j - 4) * H + (i + 1) * 128]

        store_sb = [storep.tile([128, L * 128], dt.bfloat16, tag=f"st{i}", name=f"st{i}")
                    for i in range(NI)]
        stage_sb = stagep.tile([O, L * 128], dt.bfloat16, tag="stage", name="stage")

        def vk(k):
            """input column block for kernel step k (k>=1)."""
            if k <= VHEAD:
                return vA_sb[:, (k - 1) * 128:k * 128]
            return vB_sb[:, (k - 1 - VHEAD) * 128:(k - VHEAD) * 128]

        def u_one(k, i):
            """inject u for step k, tile i (starts the PSUM accumulation)."""
            pre = prep.tile([128, 128], dt.float32, tag=f"pre{i}", name=f"pre{i}_{k}")
            nc.tensor.matmul(pre, winT_sb[:, i * 128:(i + 1) * 128], vk(k),
                             start=True, stop=False)
            return pre

        def tail_update(k, i, pre, s_prev, s_cur):
            """tanh + leaky update for tile i of step k."""
            if k >= WASH:
                m = k - WASH
                sc = store_sb[i][:, m * 128:(m + 1) * 128]
            else:
                sc = statep.tile([128, 128], dt.bfloat16, tag=f"s{i}", name=f"s{i}_{k}")
            z = zp.tile([128, 128], dt.bfloat16, tag=f"z{i}", name=f"z{i}_{k}")
            nc.scalar.activation(z, pre, Tanh)
            t01 = tp.tile([128, 128], dt.bfloat16, tag=f"t{i}", name=f"t{i}_{k}")
            nc.vector.tensor_scalar_mul(t01, s_prev[i], 0.1)
            nc.vector.tensor_add(sc, t01, z)
            s_cur.append(sc)

        # step 1 fully injected up front
        pres = [u_one(1, i) for i in range(NI)]

        s_prev = [s0_sb[:, i * 128:(i + 1) * 128] for i in range(NI)]
        for k in range(1, STEPS):
            nxt = [None] * NI
            s_cur = []
            if k == 1:
                # two sweeps so compute starts as soon as the first half of
                # W has landed (wT0) while wT1 still streams
                for i in range(NI):
                    for j in range(4):
                        nc.tensor.matmul(pres[i], wslice(j, i), s_prev[j],
                                         start=False, stop=False)
                for i in range(NI):
                    for j in range(4, NI):
                        nc.tensor.matmul(pres[i], wslice(j, i), s_prev[j],
                                         start=False, stop=(j == NI - 1))
                    tail_update(k, i, pres[i], s_prev, s_cur)
            else:
                for i in range(NI):
                    for j in range(NI):
                        nc.tensor.matmul(pres[i], wslice(j, i), s_prev[j],
                                         start=False, stop=(j == NI - 1))
                        # tile 7 of THIS step is injected here, a few slots
                        # past the tanh_7(k-1) WAR — avoids the boundary
                        # stall the end-of-step inject would pay
                        if i == 0 and j == 5 and pres[7] is None:
                            pres[7] = u_one(k, 7)
                    tail_update(k, i, pres[i], s_prev, s_cur)
            # u-injects for tiles 0-6 of step k+1 land at the step boundary;
            # tile 7 is deferred into step k+1's own group-0 emission
            if k + 1 < STEPS:
                for i in range(7):
                    nxt[i] = u_one(k + 1, i)
            pres = nxt
            s_prev = s_cur

        # ---- readout: accumulate all 8 H-tiles into [16,512] per group ----
        for g in range(8):  # 4-slot groups of 512 columns
            ro = prep.tile([O, 512], dt.float32, tag=f"pre{g}", name=f"ro{g}")
            for i in range(NI):
                nc.tensor.matmul(ro[:, :],
                                 woutT_sb[:, i * O:(i + 1) * O],
                                 store_sb[i][:, g * 512:(g + 1) * 512],
                                 start=(i == 0), stop=(i == NI - 1))
            nc.scalar.copy(stage_sb[:, g * 512:(g + 1) * 512], ro)
            if g in (3, 7):
                lo = 0 if g == 3 else 2048
                nc.sync.dma_start(qout_d[:, lo:lo + 2048],
                                  stage_sb[:, lo:lo + 2048])


def _prep_inputs(u, w, w_in, w_bias, w_out):
    """Host-side prep: per-core input maps (bf16 except host-summed output)."""
    WT = np.ascontiguousarray((A * w).T).astype(np.float32)               # [j, i]
    wTall = np.ascontiguousarray(
        WT.reshape(NI, 128, H).transpose(1, 0, 2).reshape(128, NI * H)).astype(bf16)
    win_full = np.concatenate([w_in, w_bias[:, None]], axis=1)            # [H, 65]
    winT = np.ascontiguousarray(win_full.T).astype(bf16)                  # [65, H]
    in_maps = []
    for core in range(NCORES):
        d = core // 4                       # 0 fwd, 1 bwd
        w2 = (A * w_out[1 + d * H:1 + (d + 1) * H, :]).astype(np.float32)  # [H, O]
        woutT = np.ascontiguousarray(
            w2.reshape(NI, 128, O).transpose(1, 0, 2).reshape(128, NI * O)).astype(bf16)
        v = np.zeros((STEPS, KAUG, 128), np.float32)
        ks = np.arange(STEPS)
        for b_loc in range(2):
            b = 2 * (core % 4) + b_loc
            ud = u[b] if d == 0 else u[b, ::-1]
            for c in range(C):
                ts = c * L - WASH + ks
                valid = ts >= 0
                s_idx = b_loc * C + c
                v[valid, :D, s_idx] = ud[ts[valid]]
                v[valid, D, s_idx] = 1.0
        # step 0 on host: s0 = tanh(u_proj(t0)) (zero where t0 < 0)
        up0 = np.tanh(v[0].T.astype(np.float32) @ win_full.T.astype(np.float32))  # [128, H]
        s0 = np.ascontiguousarray(
            up0.T.reshape(NI, 128, 128).transpose(1, 0, 2).reshape(128, NI * 128)
        ).astype(bf16)
        vsteps = v[1:].transpose(1, 0, 2)     # [KAUG, STEPS-1, 128]
        vA = np.ascontiguousarray(
            vsteps[:, :VHEAD].reshape(KAUG, VHEAD * 128)).astype(bf16)
        vB = np.ascontiguousarray(
            vsteps[:, VHEAD:].reshape(KAUG, (STEPS - 1 - VHEAD) * 128)).astype(bf16)
        in_maps.append({"wT0": np.ascontiguousarray(wTall[:, :4 * H]),
                        "wT1": np.ascontiguousarray(wTall[:, 4 * H:]),
                        "winT": winT, "woutT": woutT,
                        "s0": s0, "vA": vA, "vB": vB})
    return in_maps


def _assemble(results, w_out):
    y = np.zeros((B, T, O), np.float32)
    for core in range(NCORES):
        q = np.asarray(results[core]["qout"]).astype(np.float32).reshape(O, L, 128)
        d = core // 4
        for b_loc in range(2):
            b = 2 * (core % 4) + b_loc
            qq = q[:, :, b_loc * C:(b_loc + 1) * C]       # [O, L(m), C(c)]
            tmp = qq.transpose(2, 1, 0).reshape(T, O)     # t = c*L + m
            if d == 0:
                y[b] += tmp
            else:
                y[b, ::-1] += tmp
    y += w_out[0][None, None, :].astype(np.float32)
    return y


def kernel(u, w, w_in, w_bias, w_out):
    from concourse.bass_utils import run_bass_kernel_spmd

    u = np.asarray(u, np.float32)
    w = np.asarray(w, np.float32)
    w_in = np.asarray(w_in, np.float32)
    w_bias = np.asarray(w_bias, np.float32)
    w_out = np.asarray(w_out, np.float32)

    if "nc" not in _cached:
        _cached["nc"] = _build_program()
    nc = _cached["nc"]
    in_maps = _prep_inputs(u, w, w_in, w_bias, w_out)
    res = run_bass_kernel_spmd(nc, in_maps, list(range(NCORES)))
    return _assemble(res.results, w_out)


# revision 31
# speedup vs baseline: 1.0971x; 1.0063x over previous
"""Bidirectional leaky-ESN (B=8,T=2048,D=64,H=1024,O=16) on 8 TRN2 NeuronCores.

Strategy (v2)
-------------
Chunked-washout time parallelism as v1: recurrence is a contraction
(~0.56/step), so each of 16 (batch x direction) chains is split into C=64
chunks of L=32 steps run independently with a WASH=6 washout.  128
sequences per core; state transposed (H on partitions, 8 bf16 tiles
[128,128]); per step 8 u-injection matmuls (K=65) + 64 W'-stationary
matmuls accumulate into PSUM; ScalarE tanh; DVE leaky update.

v2 changes vs the 204us baseline:
- step 0 (tanh of the input projection only) is computed on the host and
  shipped as an s0 input: removes a serial ScalarE chain + 8 pairs.
- DMA split and ordered (winT, vbuf head, wT, s0, vbuf tail, woutT) so
  the u-inject stream starts ~7us earlier; ~NWARM dummy matmul pairs run
  during the DMA wait so the PE is at full p-state (2.4GHz, issue floor
  ~56ns/pair) when real work arrives instead of ramping at 1.2GHz.
- pre-activation PSUM tiles double-buffered; each step's u-injects are
  emitted after the previous step's groups, so no W matmul ever waits on
  the tanh/DVE chain (kills the ~168ns/step boundary stall).
- WASH 8 -> 6 (measured IC error well under the 2e-2 gate).
- readout runs once at the end, col-tiled: w_out^T tiles (M=16) go to
  4 concurrent 32-column PE groups (tile_position), 2 accumulated MMs per
  group -> ~4x fewer PE-serialized columns than the v1 readout; 4 partial
  strips land at PSUM partitions {0,32,64,96}+0..15 and are summed on the
  host; staged to bf16 for a 1MB output DMA.
"""

import numpy as np
import ml_dtypes

bf16 = ml_dtypes.bfloat16

B, T, D, H, O = 8, 2048, 64, 1024, 16
A = 0.9           # leaky rate
C = 64            # chunks per (batch, direction)
L = T // C        # 32 steps of real output per chunk
WASH = 5          # washout steps (step 0 runs on host)
STEPS = L + WASH
NCORES = 8
NI = H // 128     # 8 partition tiles of H
KAUG = D + 1      # 65: input dim + bias indicator row
VHEAD = 8         # steps of vbuf in the head DMA

_cached = {}


def _build_program():
    import concourse.bacc as bacc
    import concourse.mybir as mybir
    from concourse.tile import TileContext

    dt = mybir.dt
    nc = bacc.Bacc(trn_type="TRN2", target_bir_lowering=False, debug=False)

    # wTall[p, j*1024+i] = W'^T[j*128+p, i]: split j 0-3 / 4-7 so step 1
    # can start on the first half while the second streams
    wT0_d = nc.dram_tensor("wT0", [128, 4 * H], dt.bfloat16, kind="ExternalInput").ap()
    wT1_d = nc.dram_tensor("wT1", [128, 4 * H], dt.bfloat16, kind="ExternalInput").ap()
    winT_d = nc.dram_tensor("winT", [KAUG, H], dt.bfloat16, kind="ExternalInput").ap()
    woutT_d = nc.dram_tensor("woutT", [128, NI * O], dt.bfloat16, kind="ExternalInput").ap()
    s0_d = nc.dram_tensor("s0", [128, NI * 128], dt.bfloat16, kind="ExternalInput").ap()
    vA_d = nc.dram_tensor("vA", [KAUG, VHEAD * 128], dt.bfloat16, kind="ExternalInput").ap()
    vB_d = nc.dram_tensor("vB", [KAUG, (STEPS - 1 - VHEAD) * 128], dt.bfloat16,
                          kind="ExternalInput").ap()
    qout_d = nc.dram_tensor("qout", [O, L * 128], dt.bfloat16, kind="ExternalOutput").ap()

    with TileContext(nc) as tc:
        _body(tc, mybir, wT0_d, wT1_d, winT_d, woutT_d, s0_d, vA_d, vB_d, qout_d)
    nc.compile()
    return nc


def _body(tc, mybir, wT0_d, wT1_d, winT_d, woutT_d, s0_d, vA_d, vB_d, qout_d):
    dt = mybir.dt
    nc = tc.nc
    Tanh = mybir.ActivationFunctionType.Tanh

    with (
        tc.tile_pool(name="const", bufs=1) as constp,
        tc.tile_pool(name="state", bufs=3) as statep,
        tc.tile_pool(name="zp", bufs=3) as zp,
        tc.tile_pool(name="tp", bufs=3) as tp,
        tc.tile_pool(name="store", bufs=1) as storep,
        tc.tile_pool(name="stage", bufs=1) as stagep,
        tc.tile_pool(name="pre", bufs=1, space="PSUM") as prep,
    ):
        # ---- input DMAs: wT0 first (critical path), vB/woutT deferred to
        # the Scalar engine after step 1 so their packets don't steal DMA
        # bandwidth from the critical prologue transfers ----
        wT0_sb = constp.tile([128, 4 * H], dt.bfloat16, tag="wT0", name="wT0")
        nc.sync.dma_start(wT0_sb[:], wT0_d[:])
        winT_sb = constp.tile([KAUG, H], dt.bfloat16, tag="winT", name="winT")
        nc.sync.dma_start(winT_sb[:], winT_d[:])
        vA_sb = constp.tile([KAUG, VHEAD * 128], dt.bfloat16, tag="vA", name="vA")
        nc.sync.dma_start(vA_sb[:], vA_d[:])
        s0_sb = constp.tile([128, NI * 128], dt.bfloat16, tag="s0", name="s0")
        nc.sync.dma_start(s0_sb[:], s0_d[:])
        wT1_sb = constp.tile([128, 4 * H], dt.bfloat16, tag="wT1", name="wT1")
        nc.sync.dma_start(wT1_sb[:], wT1_d[:])
        vB_sb = constp.tile([KAUG, (STEPS - 1 - VHEAD) * 128], dt.bfloat16,
                            tag="vB", name="vB")
        woutT_sb = constp.tile([128, NI * O], dt.bfloat16, tag="woutT", name="woutT")

        def wslice(j, i):
            if j < 4:
                return wT0_sb[:, j * H + i * 128:j * H + (i + 1) * 128]
            return wT1_sb[:, (j - 4) * H + i * 128:(j - 4) * H + (i + 1) * 128]

        store_sb = [storep.tile([128, L * 128], dt.bfloat16, tag=f"st{i}", name=f"st{i}")
                    for i in range(NI)]
        stage_sb = stagep.tile([O, L * 128], dt.bfloat16, tag="stage", name="stage")

        def vk(k):
            """input column block for kernel step k (k>=1)."""
            if k <= VHEAD:
                return vA_sb[:, (k - 1) * 128:k * 128]
            return vB_sb[:, (k - 1 - VHEAD) * 128:(k - VHEAD) * 128]

        def u_one(k, i):
            """inject u for step k, tile i (starts the PSUM accumulation)."""
            pre = prep.tile([128, 128], dt.float32, tag=f"pre{i}", name=f"pre{i}_{k}")
            nc.tensor.matmul(pre, winT_sb[:, i * 128:(i + 1) * 128], vk(k),
                             start=True, stop=False)
            return pre

        def tail_update(k, i, pre, s_prev, s_cur):
            """tanh + leaky update for tile i of step k."""
            if k >= WASH:
                m = k - WASH
                sc = store_sb[i][:, m * 128:(m + 1) * 128]
            else:
                sc = statep.tile([128, 128], dt.bfloat16, tag=f"s{i}", name=f"s{i}_{k}")
            z = zp.tile([128, 128], dt.bfloat16, tag=f"z{i}", name=f"z{i}_{k}")
            nc.scalar.activation(z, pre, Tanh)
            t01 = tp.tile([128, 128], dt.bfloat16, tag=f"t{i}", name=f"t{i}_{k}")
            nc.vector.tensor_scalar_mul(t01, s_prev[i], 0.1)
            nc.vector.tensor_add(sc, t01, z)
            s_cur.append(sc)

        # step 1 fully injected up front
        pres = [u_one(1, i) for i in range(NI)]

        s_prev = [s0_sb[:, i * 128:(i + 1) * 128] for i in range(NI)]
        for k in range(1, STEPS):
            nxt = [None] * NI
            s_cur = []
            if k == 1:
                # two sweeps so compute starts as soon as the first half of
                # W has landed (wT0) while wT1 still streams
                for i in range(NI):
                    for j in range(4):
                        nc.tensor.matmul(pres[i], wslice(j, i), s_prev[j],
                                         start=False, stop=False)
                for i in range(NI):
                    for j in range(4, NI):
                        nc.tensor.matmul(pres[i], wslice(j, i), s_prev[j],
                                         start=False, stop=(j == NI - 1))
                    tail_update(k, i, pres[i], s_prev, s_cur)
                # non-critical input DMAs, triggered off the Scalar queue
                nc.scalar.dma_start(vB_sb[:], vB_d[:])
                nc.scalar.dma_start(woutT_sb[:], woutT_d[:])
            else:
                for i in range(NI):
                    for j in range(NI):
                        nc.tensor.matmul(pres[i], wslice(j, i), s_prev[j],
                                         start=False, stop=(j == NI - 1))
                    # tile 7 of THIS step is injected after group 0, past
                    # the tanh_7(k-1) WAR — avoids the boundary stall the
                    # end-of-step inject would pay
                    if i == 0 and pres[7] is None:
                        pres[7] = u_one(k, 7)
                    tail_update(k, i, pres[i], s_prev, s_cur)
            # u-injects for tiles 0-6 of step k+1 land at the step boundary;
            # tile 7 is deferred into step k+1's own group-0 emission
            if k + 1 < STEPS:
                for i in range(7):
                    nxt[i] = u_one(k + 1, i)
            pres = nxt
            s_prev = s_cur

        # ---- readout: accumulate all 8 H-tiles into [16,512] per group ----
        for g in range(8):  # 4-slot groups of 512 columns
            ro = prep.tile([O, 512], dt.float32, tag=f"pre{g}", name=f"ro{g}")
            for i in range(NI):
                nc.tensor.matmul(ro[:, :],
                                 woutT_sb[:, i * O:(i + 1) * O],
                                 store_sb[i][:, g * 512:(g + 1) * 512],
                                 start=(i == 0), stop=(i == NI - 1))
            nc.scalar.copy(stage_sb[:, g * 512:(g + 1) * 512], ro)
            if g % 2 == 1:
                lo = (g - 1) * 512
                nc.sync.dma_start(qout_d[:, lo:lo + 1024],
                                  stage_sb[:, lo:lo + 1024])


def _prep_inputs(u, w, w_in, w_bias, w_out):
    """Host-side prep: per-core input maps (bf16 except host-summed output)."""
    WT = np.ascontiguousarray((A * w).T).astype(np.float32)               # [j, i]
    wTall = np.ascontiguousarray(
        WT.reshape(NI, 128, H).transpose(1, 0, 2).reshape(128, NI * H)).astype(bf16)
    win_full = np.concatenate([w_in, w_bias[:, None]], axis=1)            # [H, 65]
    winT = np.ascontiguousarray(win_full.T).astype(bf16)                  # [65, H]
    in_maps = []
    for core in range(NCORES):
        d = core // 4                       # 0 fwd, 1 bwd
        w2 = (A * w_out[1 + d * H:1 + (d + 1) * H, :]).astype(np.float32)  # [H, O]
        woutT = np.ascontiguousarray(
            w2.reshape(NI, 128, O).transpose(1, 0, 2).reshape(128, NI * O)).astype(bf16)
        v = np.zeros((STEPS, KAUG, 128), np.float32)
        ks = np.arange(STEPS)
        for b_loc in range(2):
            b = 2 * (core % 4) + b_loc
            ud = u[b] if d == 0 else u[b, ::-1]
            for c in range(C):
                ts = c * L - WASH + ks
                valid = ts >= 0
                s_idx = b_loc * C + c
                v[valid, :D, s_idx] = ud[ts[valid]]
                v[valid, D, s_idx] = 1.0
        # step 0 on host: s0 = tanh(u_proj(t0)) (zero where t0 < 0)
        up0 = np.tanh(v[0].T.astype(np.float32) @ win_full.T.astype(np.float32))  # [128, H]
        s0 = np.ascontiguousarray(
            up0.T.reshape(NI, 128, 128).transpose(1, 0, 2).reshape(128, NI * 128)
        ).astype(bf16)
        vsteps = v[1:].transpose(1, 0, 2)     # [KAUG, STEPS-1, 128]
        vA = np.ascontiguousarray(
            vsteps[:, :VHEAD].reshape(KAUG, VHEAD * 128)).astype(bf16)
        vB = np.ascontiguousarray(
            vsteps[:, VHEAD:].reshape(KAUG, (STEPS - 1 - VHEAD) * 128)).astype(bf16)
        in_maps.append({"wT0": np.ascontiguousarray(wTall[:, :4 * H]),
                        "wT1": np.ascontiguousarray(wTall[:, 4 * H:]),
                        "winT": winT, "woutT": woutT,
                        "s0": s0, "vA": vA, "vB": vB})
    return in_maps


def _assemble(results, w_out):
    y = np.zeros((B, T, O), np.float32)
    for core in range(NCORES):
        q = np.asarray(results[core]["qout"]).astype(np.float32).reshape(O, L, 128)
        d = core // 4
        for b_loc in range(2):
            b = 2 * (core % 4) + b_loc
            qq = q[:, :, b_loc * C:(b_loc + 1) * C]       # [O, L(m), C(c)]
            tmp = qq.transpose(2, 1, 0).reshape(T, O)     # t = c*L + m
            if d == 0:
                y[b] += tmp
            else:
                y[b, ::-1] += tmp
    y += w_out[0][None, None, :].astype(np.float32)
    return y


def kernel(u, w, w_in, w_bias, w_out):
    from concourse.bass_utils import run_bass_kernel_spmd

    u = np.asarray(u, np.float32)
    w = np.asarray(w, np.float32)
    w_in = np.asarray(w_in, np.float32)
    w_bias = np.asarray(w_bias, np.float32)
    w_out = np.asarray(w_out, np.float32)

    if "nc" not in _cached:
        _cached["nc"] = _build_program()
    nc = _cached["nc"]
    in_maps = _prep_inputs(u, w, w_in, w_bias, w_out)
    res = run_bass_kernel_spmd(nc, in_maps, list(range(NCORES)))
    return _assemble(res.results, w_out)


# revision 34
# speedup vs baseline: 1.1356x; 1.0351x over previous
"""Bidirectional leaky-ESN (B=8,T=2048,D=64,H=1024,O=16) on 8 TRN2 NeuronCores.

Strategy (v2)
-------------
Chunked-washout time parallelism as v1: recurrence is a contraction
(~0.56/step), so each of 16 (batch x direction) chains is split into C=64
chunks of L=32 steps run independently with a WASH=6 washout.  128
sequences per core; state transposed (H on partitions, 8 bf16 tiles
[128,128]); per step 8 u-injection matmuls (K=65) + 64 W'-stationary
matmuls accumulate into PSUM; ScalarE tanh; DVE leaky update.

v2 changes vs the 204us baseline:
- step 0 (tanh of the input projection only) is computed on the host and
  shipped as an s0 input: removes a serial ScalarE chain + 8 pairs.
- DMA split and ordered (winT, vbuf head, wT, s0, vbuf tail, woutT) so
  the u-inject stream starts ~7us earlier; ~NWARM dummy matmul pairs run
  during the DMA wait so the PE is at full p-state (2.4GHz, issue floor
  ~56ns/pair) when real work arrives instead of ramping at 1.2GHz.
- pre-activation PSUM tiles double-buffered; each step's u-injects are
  emitted after the previous step's groups, so no W matmul ever waits on
  the tanh/DVE chain (kills the ~168ns/step boundary stall).
- WASH 8 -> 6 (measured IC error well under the 2e-2 gate).
- readout runs once at the end, col-tiled: w_out^T tiles (M=16) go to
  4 concurrent 32-column PE groups (tile_position), 2 accumulated MMs per
  group -> ~4x fewer PE-serialized columns than the v1 readout; 4 partial
  strips land at PSUM partitions {0,32,64,96}+0..15 and are summed on the
  host; staged to bf16 for a 1MB output DMA.
"""

import numpy as np
import ml_dtypes

bf16 = ml_dtypes.bfloat16

B, T, D, H, O = 8, 2048, 64, 1024, 16
A = 0.9           # leaky rate
C = 64            # chunks per (batch, direction)
L = T // C        # 32 steps of real output per chunk
WASH = 5          # washout steps (step 0 runs on host)
STEPS = L + WASH
NCORES = 8
NI = H // 128     # 8 partition tiles of H
KAUG = D + 1      # 65: input dim + bias indicator row
VHEAD = 8         # steps of vbuf in the head DMA

_cached = {}


def _build_program():
    import concourse.bacc as bacc
    import concourse.mybir as mybir
    from concourse.tile import TileContext

    dt = mybir.dt
    nc = bacc.Bacc(trn_type="TRN2", target_bir_lowering=False, debug=False)

    # wTall[p, j*1024+i] = W'^T[j*128+p, i]: split j 0-3 / 4-7 so step 1
    # can start on the first half while the second streams
    wT0_d = nc.dram_tensor("wT0", [128, 4 * H], dt.bfloat16, kind="ExternalInput").ap()
    wT1_d = nc.dram_tensor("wT1", [128, 4 * H], dt.bfloat16, kind="ExternalInput").ap()
    winT_d = nc.dram_tensor("winT", [KAUG, H], dt.bfloat16, kind="ExternalInput").ap()
    woutT_d = nc.dram_tensor("woutT", [128, NI * O], dt.bfloat16, kind="ExternalInput").ap()
    s0_d = nc.dram_tensor("s0", [128, NI * 128], dt.bfloat16, kind="ExternalInput").ap()
    vA_d = nc.dram_tensor("vA", [KAUG, VHEAD * 128], dt.bfloat16, kind="ExternalInput").ap()
    vB_d = nc.dram_tensor("vB", [KAUG, (STEPS - 1 - VHEAD) * 128], dt.bfloat16,
                          kind="ExternalInput").ap()
    qout_d = nc.dram_tensor("qout", [O, L * 128], dt.bfloat16, kind="ExternalOutput").ap()

    with TileContext(nc) as tc:
        _body(tc, mybir, wT0_d, wT1_d, winT_d, woutT_d, s0_d, vA_d, vB_d, qout_d)
    nc.compile()
    return nc


def _body(tc, mybir, wT0_d, wT1_d, winT_d, woutT_d, s0_d, vA_d, vB_d, qout_d):
    dt = mybir.dt
    nc = tc.nc
    Tanh = mybir.ActivationFunctionType.Tanh

    with (
        tc.tile_pool(name="const", bufs=1) as constp,
        tc.tile_pool(name="state", bufs=3) as statep,
        tc.tile_pool(name="zp", bufs=3) as zp,
        tc.tile_pool(name="tp", bufs=3) as tp,
        tc.tile_pool(name="store", bufs=1) as storep,
        tc.tile_pool(name="stage", bufs=1) as stagep,
        tc.tile_pool(name="pre", bufs=1, space="PSUM") as prep,
    ):
        # ---- input DMAs: wT0 first (critical path — its packets queue
        # ahead in every DMA engine), then the small early tensors, then
        # the rest in order of first use ----
        wT0_sb = constp.tile([128, 4 * H], dt.bfloat16, tag="wT0", name="wT0")
        nc.sync.dma_start(wT0_sb[:], wT0_d[:])
        winT_sb = constp.tile([KAUG, H], dt.bfloat16, tag="winT", name="winT")
        nc.sync.dma_start(winT_sb[:], winT_d[:])
        vA_sb = constp.tile([KAUG, VHEAD * 128], dt.bfloat16, tag="vA", name="vA")
        nc.sync.dma_start(vA_sb[:], vA_d[:])
        s0_sb = constp.tile([128, NI * 128], dt.bfloat16, tag="s0", name="s0")
        nc.sync.dma_start(s0_sb[:], s0_d[:])
        wT1_sb = constp.tile([128, 4 * H], dt.bfloat16, tag="wT1", name="wT1")
        nc.sync.dma_start(wT1_sb[:], wT1_d[:])
        vB_sb = constp.tile([KAUG, (STEPS - 1 - VHEAD) * 128], dt.bfloat16,
                            tag="vB", name="vB")
        nc.sync.dma_start(vB_sb[:], vB_d[:])
        woutT_sb = constp.tile([128, NI * O], dt.bfloat16, tag="woutT", name="woutT")
        nc.sync.dma_start(woutT_sb[:], woutT_d[:])

        def wslice(j, i):
            if j < 4:
                return wT0_sb[:, j * H + i * 128:j * H + (i + 1) * 128]
            return wT1_sb[:, (j - 4) * H + i * 128:(j - 4) * H + (i + 1) * 128]

        store_sb = [storep.tile([128, L * 128], dt.bfloat16, tag=f"st{i}", name=f"st{i}")
                    for i in range(NI)]
        stage_sb = stagep.tile([O, L * 128], dt.bfloat16, tag="stage", name="stage")

        def vk(k):
            """input column block for kernel step k (k>=1)."""
            if k <= VHEAD:
                return vA_sb[:, (k - 1) * 128:k * 128]
            return vB_sb[:, (k - 1 - VHEAD) * 128:(k - VHEAD) * 128]

        def u_one(k, i):
            """inject u for step k, tile i (starts the PSUM accumulation)."""
            pre = prep.tile([128, 128], dt.float32, tag=f"pre{i}", name=f"pre{i}_{k}")
            nc.tensor.matmul(pre, winT_sb[:, i * 128:(i + 1) * 128], vk(k),
                             start=True, stop=False)
            return pre

        def tail_update(k, i, pre, s_prev, s_cur):
            """tanh + leaky update for tile i of step k."""
            if k >= WASH:
                m = k - WASH
                sc = store_sb[i][:, m * 128:(m + 1) * 128]
            else:
                sc = statep.tile([128, 128], dt.bfloat16, tag=f"s{i}", name=f"s{i}_{k}")
            z = zp.tile([128, 128], dt.bfloat16, tag=f"z{i}", name=f"z{i}_{k}")
            nc.scalar.activation(z, pre, Tanh)
            t01 = tp.tile([128, 128], dt.bfloat16, tag=f"t{i}", name=f"t{i}_{k}")
            nc.vector.tensor_scalar_mul(t01, s_prev[i], 0.1)
            nc.vector.tensor_add(sc, t01, z)
            s_cur.append(sc)

        # step 1 fully injected up front
        pres = [u_one(1, i) for i in range(NI)]

        s_prev = [s0_sb[:, i * 128:(i + 1) * 128] for i in range(NI)]
        for k in range(1, STEPS):
            nxt = [None] * NI
            s_cur = []
            if k == 1:
                # two sweeps so compute starts as soon as the first half of
                # W has landed (wT0) while wT1 still streams
                for i in range(NI):
                    for j in range(4):
                        nc.tensor.matmul(pres[i], wslice(j, i), s_prev[j],
                                         start=False, stop=False)
                for i in range(NI):
                    for j in range(4, NI):
                        nc.tensor.matmul(pres[i], wslice(j, i), s_prev[j],
                                         start=False, stop=(j == NI - 1))
                    tail_update(k, i, pres[i], s_prev, s_cur)
            else:
                # group 0 with deferred j=7: s_cur[7] of step k-1 is only
                # ready ~1us past the boundary, so group 0's last pair is
                # pushed to slot ~19 by interleaving the u7 inject and
                # group 1's first pairs ahead of it
                for j in range(7):
                    nc.tensor.matmul(pres[0], wslice(j, 0), s_prev[j],
                                     start=False, stop=False)
                if pres[7] is None:
                    pres[7] = u_one(k, 7)
                for j in range(3):
                    nc.tensor.matmul(pres[1], wslice(j, 1), s_prev[j],
                                     start=False, stop=False)
                nc.tensor.matmul(pres[0], wslice(7, 0), s_prev[7],
                                 start=False, stop=True)
                tail_update(k, 0, pres[0], s_prev, s_cur)
                for j in range(3, NI):
                    nc.tensor.matmul(pres[1], wslice(j, 1), s_prev[j],
                                     start=False, stop=(j == NI - 1))
                tail_update(k, 1, pres[1], s_prev, s_cur)
                for i in range(2, NI):
                    for j in range(NI):
                        nc.tensor.matmul(pres[i], wslice(j, i), s_prev[j],
                                         start=False, stop=(j == NI - 1))
                    tail_update(k, i, pres[i], s_prev, s_cur)
            # u-injects for tiles 0-6 of step k+1 land at the step boundary;
            # tile 7 is deferred into step k+1's own group-0 emission
            if k + 1 < STEPS:
                for i in range(7):
                    nxt[i] = u_one(k + 1, i)
            pres = nxt
            s_prev = s_cur

        # ---- readout: accumulate all 8 H-tiles into [16,512] per group ----
        for g in range(8):  # 4-slot groups of 512 columns
            ro = prep.tile([O, 512], dt.float32, tag=f"pre{g}", name=f"ro{g}")
            for i in range(NI):
                nc.tensor.matmul(ro[:, :],
                                 woutT_sb[:, i * O:(i + 1) * O],
                                 store_sb[i][:, g * 512:(g + 1) * 512],
                                 start=(i == 0), stop=(i == NI - 1))
            nc.scalar.copy(stage_sb[:, g * 512:(g + 1) * 512], ro)
            if g % 2 == 1:
                lo = (g - 1) * 512
                nc.sync.dma_start(qout_d[:, lo:lo + 1024],
                                  stage_sb[:, lo:lo + 1024])


def _prep_inputs(u, w, w_in, w_bias, w_out):
    """Host-side prep: per-core input maps (bf16 except host-summed output)."""
    WT = np.ascontiguousarray((A * w).T).astype(np.float32)               # [j, i]
    wTall = np.ascontiguousarray(
        WT.reshape(NI, 128, H).transpose(1, 0, 2).reshape(128, NI * H)).astype(bf16)
    win_full = np.concatenate([w_in, w_bias[:, None]], axis=1)            # [H, 65]
    winT = np.ascontiguousarray(win_full.T).astype(bf16)                  # [65, H]
    in_maps = []
    for core in range(NCORES):
        d = core // 4                       # 0 fwd, 1 bwd
        w2 = (A * w_out[1 + d * H:1 + (d + 1) * H, :]).astype(np.float32)  # [H, O]
        woutT = np.ascontiguousarray(
            w2.reshape(NI, 128, O).transpose(1, 0, 2).reshape(128, NI * O)).astype(bf16)
        v = np.zeros((STEPS, KAUG, 128), np.float32)
        ks = np.arange(STEPS)
        for b_loc in range(2):
            b = 2 * (core % 4) + b_loc
            ud = u[b] if d == 0 else u[b, ::-1]
            for c in range(C):
                ts = c * L - WASH + ks
                valid = ts >= 0
                s_idx = b_loc * C + c
                v[valid, :D, s_idx] = ud[ts[valid]]
                v[valid, D, s_idx] = 1.0
        # step 0 on host: s0 = tanh(u_proj(t0)) (zero where t0 < 0)
        up0 = np.tanh(v[0].T.astype(np.float32) @ win_full.T.astype(np.float32))  # [128, H]
        s0 = np.ascontiguousarray(
            up0.T.reshape(NI, 128, 128).transpose(1, 0, 2).reshape(128, NI * 128)
        ).astype(bf16)
        vsteps = v[1:].transpose(1, 0, 2)     # [KAUG, STEPS-1, 128]
        vA = np.ascontiguousarray(
            vsteps[:, :VHEAD].reshape(KAUG, VHEAD * 128)).astype(bf16)
        vB = np.ascontiguousarray(
            vsteps[:, VHEAD:].reshape(KAUG, (STEPS - 1 - VHEAD) * 128)).astype(bf16)
        in_maps.append({"wT0": np.ascontiguousarray(wTall[:, :4 * H]),
                        "wT1": np.ascontiguousarray(wTall[:, 4 * H:]),
                        "winT": winT, "woutT": woutT,
                        "s0": s0, "vA": vA, "vB": vB})
    return in_maps


def _assemble(results, w_out):
    y = np.zeros((B, T, O), np.float32)
    for core in range(NCORES):
        q = np.asarray(results[core]["qout"]).astype(np.float32).reshape(O, L, 128)
        d = core // 4
        for b_loc in range(2):
            b = 2 * (core % 4) + b_loc
            qq = q[:, :, b_loc * C:(b_loc + 1) * C]       # [O, L(m), C(c)]
            tmp = qq.transpose(2, 1, 0).reshape(T, O)     # t = c*L + m
            if d == 0:
                y[b] += tmp
            else:
                y[b, ::-1] += tmp
    y += w_out[0][None, None, :].astype(np.float32)
    return y


def kernel(u, w, w_in, w_bias, w_out):
    from concourse.bass_utils import run_bass_kernel_spmd

    u = np.asarray(u, np.float32)
    w = np.asarray(w, np.float32)
    w_in = np.asarray(w_in, np.float32)
    w_bias = np.asarray(w_bias, np.float32)
    w_out = np.asarray(w_out, np.float32)

    if "nc" not in _cached:
        _cached["nc"] = _build_program()
    nc = _cached["nc"]
    in_maps = _prep_inputs(u, w, w_in, w_bias, w_out)
    res = run_bass_kernel_spmd(nc, in_maps, list(range(NCORES)))
    return _assemble(res.results, w_out)
